# revision 79
# baseline (speedup 1.0000x reference)
"""Block-causal GQA attention layer on 8 Trainium2 NeuronCores.

Sharding: 8 cores = batch(2) x head-group(4). Core c handles batch b=c//4 and
head group g=c%4 (q heads 4g..4g+3, kv head g). W_attn is column-sharded by
head group, W_proj row-sharded; each core computes a partial [T, C] output and
the host sums the 4 partials per batch element.

All on-device tensors are bf16 (matmuls run 1 cycle/row at any width; DVE gets
2x on 16-bit; DMA traffic halves) except PSUM accumulators and the final
output, which stay f32. x is pre-transposed on the host (xT [C,T]) so the
kernel needs no PE transposes for the QKV input. RoPE cos/sin tables (with the
q/k norm weights folded in) are packed [T, 4, HD] so each chunk is one DMA.

Schedule (single PE stream, engines pipelined via Tile deps):
  B) 16 t-chunks: iter i does xT/table DMAs, QKV matmuls into PSUM, a bf16
     SBUF copy of q/k (ACT; DVE for the first chunks while wa dma_starts
     clog the ACT sequencer), RMS square-sums (ACT Square with accum), and an
     all-DVE rsqrt (reciprocal_approx_fast seed + 2 Newton steps — keeping
     Sqrt off ACT holds the activation table on the one set that also has
     Exp, avoiding ~1.3us reloads at every interleave point). Lag-1 RoPE
     (Pool+DVE, rs applied by a final per-head tensor_scalar), lag-2 PE
     transposes of q/k into bf16 PSUM, DVE copyback into qT/kT.
     The Ti=0 block and all of Ti=1 interleave into iters 9..15 + epilogue:
     they are exp(ACT)-bound and fill B's ACT slack, and the (1,2)/(1,3)
     blocks keep PE busy through the chunk-15 drain (no chunk-15 deps).
  C) Ti=2,3 x head h: block-causal scores (full-width chunks exp'd in PSUM
     pairs, diag r=1,2 share a pair via a union-region exp; exact 128-wide
     staircase offsets), mask multiplies on the leading 128 cols, PV
     accumulation; softmax denominator = DVE binary-tree sum of the
     full-width exp chunks + one ones-matmul, plus 3 narrow f32 ones-matmuls
     for the dominant diagonal chunks (keeps output error at 6.8e-3),
     reciprocal_approx_fast, normalize into bf16 yT (finish lagged one head
     so PE never waits on the tree). Projection quarter-blocks interleave
     into every head slot (2 per Ti=2 slot, 1 per Ti=3 slot, 4 in the drain
     tail); PSUM->SBUF copies split 3:1 ACT:DVE (GPSIMD cannot access
     PSUM), one 8KB-run out DMA per t-chunk, split finer in the drain tail.

DMA queues: xT + out on SP, wa + wp on ACT, tables + mask on gpsimd SWDGE —
a single queue's ~1.3us per-dma_start sequencer cost would starve the rest.
"""

import numpy as np
import ml_dtypes

import concourse.bacc as bacc
import concourse.bass as bass
import concourse.tile as tile
import concourse.mybir as mybir
from concourse.bass_utils import run_bass_kernel_spmd
from concourse.masks import make_identity

P = 128
T = 2048
C = 2048
N_HEAD = 16
N_KV = 4
HD = 128          # head dim
HG = N_HEAD // N_KV  # heads per group = 4
BLOCK = 16
EPS = 1e-5
ROPE_BASE = 500000.0
QCOLS = HG * HD   # 512 q cols per core
JCOLS = QCOLS + 2 * HD  # 768 qkv cols per core
NT = T // P       # 16 t-chunks
NC16 = C // P     # 16 c-chunks
NX = 8            # xT DMA chunks (256 t-cols each)
SCALE = 1.0 / float(np.sqrt(np.float32(HD)))

F32 = mybir.dt.float32
BF = mybir.dt.bfloat16
AF = mybir.ActivationFunctionType
ALU = mybir.AluOpType
BF_NP = ml_dtypes.bfloat16


def build_nc():
    nc = bacc.Bacc("TRN2", target_bir_lowering=False)

    xt = nc.dram_tensor("xt", [C, T], BF, kind="ExternalInput")
    wa = nc.dram_tensor("wa", [C, JCOLS], BF, kind="ExternalInput")
    wp = nc.dram_tensor("wp", [QCOLS, C], BF, kind="ExternalInput")
    tabs = nc.dram_tensor("tabs", [T, 4, HD], BF, kind="ExternalInput")
    dm1 = nc.dram_tensor("dm1", [P, P], BF, kind="ExternalInput")
    out = nc.dram_tensor("out", [T, C], F32, kind="ExternalOutput")

    with tile.TileContext(nc) as tc:
        with tc.tile_pool(name="persist", bufs=1) as persist:
            ident_f = persist.tile([P, P], F32)
            make_identity(nc, ident_f)
            identb = persist.tile([P, P], BF)
            nc.vector.tensor_copy(identb, ident_f)
            ones_f = persist.tile([P, P], F32)
            nc.vector.memset(ones_f, 1.0)
            onesb = persist.tile([P, P], BF)
            nc.vector.tensor_copy(onesb, ones_f)
            dm1_sb = persist.tile([P, P], BF)
            nc.gpsimd.dma_start(dm1_sb, dm1[:])
            eps_sb = persist.tile([P, 1], F32)
            nc.vector.memset(eps_sb, EPS)

            qT = persist.tile([P, HG, T], BF)     # [d, h, t]
            kT = persist.tile([P, T], BF)         # [d, t]
            v_sb = persist.tile([P, NT, HD], BF)  # [s_in_chunk, s_chunk, d']
            yT = persist.tile([P, HG, T], BF)     # [d', h, t]
            wp_sb = persist.tile([P, HG, C], BF)

            # ---------------- Phase B (pipelined, lag 1) + C(Ti=0) --------
            with (
                tc.tile_pool(name="psB_qa", bufs=2, space="PSUM") as psB_qa,
                tc.tile_pool(name="psB_qb", bufs=2, space="PSUM") as psB_qb,
                tc.tile_pool(name="psC0_sc", bufs=2, space="PSUM") as psC0_sc,
                tc.tile_pool(name="psC0_yt", bufs=1, space="PSUM") as psC0_yt,
                tc.tile_pool(name="psB_tp", bufs=1, space="PSUM") as psB_tp,
                tc.tile_pool(name="wts", bufs=1) as wts,
                tc.tile_pool(name="bstream", bufs=2) as bstream,
                tc.tile_pool(name="tstream", bufs=4) as tstream,
                tc.tile_pool(name="bwork", bufs=4) as bwork,
                tc.tile_pool(name="c0work", bufs=2) as c0work,
            ):
                half = HD // 2
                st = {}        # chunk index -> per-chunk state dict
                x_tiles = {}   # xchunk index -> [P, NC16, 2P] tile

                xt_r = xt[:].rearrange("(ci p) t -> p ci t", p=P)

                def dma_xw(j, split=False):
                    xw = bstream.tile([P, NC16, 2 * P], BF, tag="xw")
                    if split:  # halve first-chunk latency
                        nc.sync.dma_start(
                            xw[:, 0:8, :], xt_r[:, 0:8, j * 2 * P : (j + 1) * 2 * P]
                        )
                        nc.sync.dma_start(
                            xw[:, 8:16, :], xt_r[:, 8:16, j * 2 * P : (j + 1) * 2 * P]
                        )
                    else:
                        nc.sync.dma_start(xw, xt_r[:, :, j * 2 * P : (j + 1) * 2 * P])
                    x_tiles[j] = xw

                wa_r = wa[:].rearrange("(co ci) j -> ci co j", ci=P)
                wa_tiles = []

                def load_wa():
                    for ci in range(NC16):
                        wa_ci = wts.tile([P, JCOLS], BF, tag=f"wa{ci}", name=f"wa{ci}")
                        nc.scalar.dma_start(wa_ci, wa_r[:, ci])
                        wa_tiles.append(wa_ci)

                def wa_tile(ci):
                    return wa_tiles[ci]

                tab_tiles = {}

                def dma_tab(i):
                    tab = tstream.tile([P, 4, HD], BF, tag="tab", name="tab_t")
                    nc.gpsimd.dma_start(tab, tabs[i * P : (i + 1) * P])
                    tab_tiles[i] = tab

                def stageB1(j):
                    """Pool/DVE rope for chunk j on the bf16 SBUF copy of the
                    QKV output (GPSIMD cannot touch PSUM and only supports
                    plain tensor_tensor); the RMS scale rs is applied by the
                    final per-head DVE tensor_scalar."""
                    s = st[j]
                    qsb, rs, tab = s["qsb"], s["rs"], s["tab"]

                    csq = tab[:, 0, :]
                    snq = tab[:, 1, :]
                    csk = tab[:, 2, :]
                    snk = tab[:, 3, :]
                    csq_b = bass.AP(
                        tensor=csq.tensor,
                        offset=csq.offset,
                        ap=[csq.ap[0], [0, HG], [1, HD]],
                    )
                    snq_b = bass.AP(
                        tensor=snq.tensor,
                        offset=snq.offset,
                        ap=[snq.ap[0], [0, HG], [half, 2], [1, half]],
                    )
                    qswp = bass.AP(
                        tensor=qsb.tensor,
                        offset=qsb.offset + half,
                        ap=[qsb.ap[0], [HD, HG], [-half, 2], [1, half]],
                    )
                    t1q = bwork.tile([P, QCOLS], BF, tag="t1q")
                    nc.gpsimd.tensor_tensor(
                        t1q.rearrange("p (h e) -> p h e", h=HG),
                        qsb[:, 0:QCOLS].rearrange("p (h e) -> p h e", h=HG),
                        csq_b,
                        ALU.mult,
                    )
                    t2q = bwork.tile([P, QCOLS], BF, tag="t2q")
                    nc.vector.tensor_tensor(
                        t2q.rearrange("p (h s e) -> p h s e", h=HG, s=2),
                        qswp,
                        snq_b,
                        ALU.mult,
                    )
                    qsum = bwork.tile([P, QCOLS], BF, tag="qsum")
                    nc.vector.tensor_tensor(qsum, t1q, t2q, ALU.add)
                    qhat = bwork.tile([P, QCOLS], BF, tag="qhat")
                    for hh in range(HG):
                        o = hh * HD
                        nc.vector.tensor_scalar_mul(
                            qhat[:, o : o + HD], qsum[:, o : o + HD],
                            rs[:, hh : hh + 1],
                        )

                    t1k = bwork.tile([P, HD], BF, tag="t1k")
                    nc.gpsimd.tensor_tensor(
                        t1k, qsb[:, QCOLS : QCOLS + HD], csk, ALU.mult
                    )
                    kswp = bass.AP(
                        tensor=qsb.tensor,
                        offset=qsb.offset + QCOLS + half,
                        ap=[qsb.ap[0], [-half, 2], [1, half]],
                    )
                    t2k = bwork.tile([P, HD], BF, tag="t2k")
                    nc.vector.tensor_tensor(
                        t2k.rearrange("p (s e) -> p s e", s=2),
                        kswp,
                        snk.rearrange("p (s e) -> p s e", s=2),
                        ALU.mult,
                    )
                    ksum = bwork.tile([P, HD], BF, tag="ksum")
                    nc.vector.tensor_tensor(ksum, t1k, t2k, ALU.add)
                    khat = bwork.tile([P, HD], BF, tag="khat")
                    nc.vector.tensor_scalar_mul(khat, ksum, rs[:, HG : HG + 1])
                    s["qhat"], s["khat"] = qhat, khat

                def stageB2(j):
                    """PE transposes of qhat/khat (bf16 PSUM) + copyback."""
                    s = st.pop(j)
                    t0 = j * P
                    tqk_ps = psB_tp.tile([P, 640], BF, tag="tp")
                    for hh in range(HG):
                        nc.tensor.transpose(
                            tqk_ps[:, hh * HD : (hh + 1) * HD],
                            s["qhat"][:, hh * HD : (hh + 1) * HD],
                            identb,
                        )
                    nc.tensor.transpose(tqk_ps[:, QCOLS : QCOLS + HD], s["khat"], identb)
                    nc.vector.tensor_copy(
                        qT[:, :, t0 : t0 + P],
                        tqk_ps[:, 0:QCOLS].rearrange("p (h t) -> p h t", h=HG),
                    )
                    nc.vector.tensor_copy(kT[:, t0 : t0 + P], tqk_ps[:, QCOLS:640])

                # --- early attention blocks (Ti=0 all h, Ti=1 h=0,1),
                # interleaved into B's tail iterations: they fill B's ACT/PE
                # slack, keep PE busy through the B epilogue, and shrink the
                # serial C region. Unpaired exps, c0 PSUM pools.
                c0state = {}

                def b_attn_block(Ti, h):
                    tt0 = Ti * 512
                    nS = 4 * Ti + 4
                    nfull = 4 * Ti + 1
                    exa = c0work.tile([P, 8, 512], BF, tag="c0ex")
                    yt_ps = psC0_yt.tile([P, 512], F32, tag="c0yt")
                    for S in range(nS):
                        r = S - 4 * Ti
                        off = 128 * r if r > 0 else 0
                        sc_ps = psC0_sc.tile([P, 512], F32, tag="c0sc")
                        nc.tensor.matmul(
                            sc_ps[:, off:512],
                            kT[:, S * P : (S + 1) * P],
                            qT[:, h, tt0 + off : tt0 + 512],
                            start=True,
                            stop=True,
                        )
                        nc.scalar.activation(
                            exa[:, S, off:512], sc_ps[:, off:512], AF.Exp,
                            scale=SCALE,
                        )
                        if r >= 0:
                            nc.vector.tensor_tensor(
                                exa[:, S, off : off + P],
                                exa[:, S, off : off + P],
                                dm1_sb,
                                ALU.mult,
                            )
                        nc.tensor.matmul(
                            yt_ps[:, off:512],
                            v_sb[:, S, :],
                            exa[:, S, off:512],
                            start=(S == 0),
                            stop=(S == nS - 1),
                        )
                    gap = 1
                    while gap < nfull:
                        for k in range(0, nfull - gap, 2 * gap):
                            nc.vector.tensor_tensor(
                                exa[:, k, :], exa[:, k, :],
                                exa[:, k + gap, :], ALU.add,
                            )
                        gap *= 2
                    c0state[(Ti, h)] = (exa, yt_ps)

                def b_attn_finish(Ti, h):
                    exa, yt_ps = c0state.pop((Ti, h))
                    tt0 = Ti * 512
                    den_ps = psC0_sc.tile([P, 512], F32, tag="c0sc", name="den0")
                    nc.tensor.matmul(den_ps, onesb, exa[:, 0, :], start=True, stop=False)
                    for r in range(1, 4):
                        off = 128 * r
                        nc.tensor.matmul(
                            den_ps[:, off:512],
                            onesb,
                            exa[:, 4 * Ti + r, off:512],
                            start=False,
                            stop=(r == 3),
                        )
                    denr = c0work.tile([P, 512], F32, tag="c0denr")
                    nc.vector.reciprocal_approx_fast(denr, den_ps)
                    nc.vector.tensor_tensor(
                        yT[:, h, tt0 : tt0 + 512], yt_ps, denr, ALU.mult
                    )

                dma_xw(0, split=True)
                dma_tab(0)
                load_wa()
                dma_tab(1)
                for i in range(NT + 1):
                    if i < NT:
                        if i % 2 == 0 and i // 2 + 1 < NX:
                            dma_xw(i // 2 + 1)
                        if i + 2 < NT:
                            dma_tab(i + 2)
                        if i == 8:
                            nc.scalar.dma_start(
                                wp_sb, wp[:].rearrange("(h d) e -> d h e", d=P)
                            )

                        # premult + rope for chunk i-1 (ACT/DVE overlap the MMs)
                        if i >= 1:
                            stageB1(i - 1)

                        # QKV matmuls: iter 0 emits chunks 0 AND 1 with the
                        # ci loops interleaved, so PE consumption of each wa
                        # tile (4 matmuls) matches its DMA arrival rate and
                        # the cold-start weight trickle never idles PE
                        def emit_qkv(chunks):
                            tiles = {}
                            for j in chunks:
                                sj = st.setdefault(j, {})
                                sj["tab"] = tab_tiles[j]
                                qa = psB_qa.tile([P, QCOLS], F32, tag="qa")
                                qb = psB_qb.tile([P, 2 * HD], F32, tag="qb")
                                sj["qa"], sj["qb"] = qa, qb
                                tiles[j] = (qa, qb)
                            for ci in range(NC16):
                                wac = wa_tile(ci)
                                for j in chunks:
                                    qa, qb = tiles[j]
                                    lhsT = x_tiles[j // 2][
                                        :, ci, (j % 2) * P : (j % 2) * P + P
                                    ]
                                    nc.tensor.matmul(
                                        qa, lhsT, wac[:, 0:QCOLS],
                                        start=(ci == 0), stop=(ci == NC16 - 1),
                                    )
                                    nc.tensor.matmul(
                                        qb, lhsT, wac[:, QCOLS:JCOLS],
                                        start=(ci == 0), stop=(ci == NC16 - 1),
                                    )

                        def emit_stats(j):
                            s = st[j]
                            qa_ps, qb_ps = s["qa"], s["qb"]
                            # v copy + bf16 SBUF copy of q/k (rope reads it
                            # from Pool, which cannot access PSUM) + square
                            # sums, on ACT (Square/Copy share the Exp
                            # act-table set; the first chunks' copies go to
                            # DVE — the wa dma_starts occupy the ACT
                            # sequencer for ~20us at cold start).
                            early = j < 3
                            NH1 = HG + 1
                            ss = bwork.tile([P, NH1], F32, tag="ss")
                            qsb = bwork.tile([P, QCOLS + HD], BF, tag="qsb")
                            if early:
                                nc.vector.tensor_copy(
                                    v_sb[:, j, :], qb_ps[:, HD : 2 * HD]
                                )
                                nc.vector.tensor_copy(qsb[:, 0:QCOLS], qa_ps)
                                nc.vector.tensor_copy(
                                    qsb[:, QCOLS : QCOLS + HD], qb_ps[:, 0:HD]
                                )
                            else:
                                nc.scalar.copy(
                                    v_sb[:, j, :], qb_ps[:, HD : 2 * HD]
                                )
                                nc.scalar.copy(qsb[:, 0:QCOLS], qa_ps)
                                nc.scalar.copy(
                                    qsb[:, QCOLS : QCOLS + HD], qb_ps[:, 0:HD]
                                )
                            s["qsb"] = qsb
                            for hh in range(NH1):
                                sq = bwork.tile([P, HD], F32, tag="sq")
                                src = (
                                    qa_ps[:, hh * HD : (hh + 1) * HD]
                                    if hh < HG
                                    else qb_ps[:, 0:HD]
                                )
                                nc.scalar.activation(
                                    sq, src, AF.Square,
                                    accum_out=ss[:, hh : hh + 1],
                                )
                            # rs = (eps + ss/HD)^-1/2 on DVE: reciprocal seed
                            # + 2 Newton steps (y' = y*(1.5 - 0.5*m*y^2)).
                            # Keeping Sqrt off ACT avoids act-table reloads at
                            # every B<->attention interleave point (~1.3us).
                            m_t = bwork.tile([P, NH1], F32, tag="m_t")
                            nc.vector.tensor_scalar(
                                m_t, ss, 1.0 / HD, EPS, ALU.mult, ALU.add
                            )
                            rr = bwork.tile([P, NH1], F32, tag="rr")
                            nc.vector.reciprocal_approx_fast(rr, m_t)
                            rs = bwork.tile([P, NH1], F32, tag="rs")
                            nc.vector.tensor_scalar(
                                rs, rr, 0.5, 0.5, ALU.mult, ALU.add
                            )
                            nr1 = bwork.tile([P, NH1], F32, tag="nr1")
                            nr2 = bwork.tile([P, NH1], F32, tag="nr2")
                            for _ in range(2):
                                nc.vector.tensor_tensor(nr1, m_t, rs, ALU.mult)
                                nc.vector.scalar_tensor_tensor(
                                    nr2, nr1, -0.5, rs, ALU.mult, ALU.mult
                                )
                                nc.vector.scalar_tensor_tensor(
                                    rs, nr2, 1.5, rs, ALU.add, ALU.mult
                                )
                            s["rs"] = rs

                        emit_qkv([i])
                        # chunk i-2 q/k transposes (lag 2: qhat is then
                        # guaranteed ready, PE never waits on rope)
                        if i >= 2:
                            stageB2(i - 2)
                        emit_stats(i)

                        # early attention interleave: all of Ti=0 and Ti=1
                        if 10 <= i <= 13:
                            b_attn_finish(0, i - 10)
                        elif i == 14:
                            b_attn_finish(1, 0)
                        elif i == 15:
                            b_attn_finish(1, 1)
                        if 9 <= i <= 12:
                            b_attn_block(0, i - 9)
                        elif i == 13:
                            b_attn_block(1, 0)
                        elif i == 14:
                            b_attn_block(1, 1)
                        elif i == 15:
                            b_attn_block(1, 2)
                    else:
                        # epilogue: chunk 15 rope/transposes interleaved with
                        # the (1,3) block (which has no chunk-15 dependency)
                        # so PE stays busy while ACT/DVE drain
                        stageB1(i - 1)
                        stageB2(i - 2)
                        b_attn_finish(1, 2)
                        b_attn_block(1, 3)
                        stageB2(i - 1)
                        b_attn_finish(1, 3)

            # ---------------- Phase C: Ti=1..3 + projection --------------
            with (
                tc.tile_pool(name="cwork", bufs=2) as cwork,
                tc.tile_pool(name="cfin", bufs=2) as cfin,
                tc.tile_pool(name="dout", bufs=3) as dout,
                tc.tile_pool(name="psC_yt", bufs=2, space="PSUM") as psC_yt,
                tc.tile_pool(name="psC_scr", bufs=2, space="PSUM") as psC_scr,
                tc.tile_pool(name="psC_sc", bufs=2, space="PSUM") as psC_sc,
            ):
                def emit_proj_part(Tb, part, tail=False, last=False):
                    tci = 4 * Tb + part
                    t0 = tci * P
                    o_sb = dout.tile([P, C], F32, tag="o_sb")
                    for e in range(4):
                        # in the drain tail, spread PSUM slots and copies
                        # across idle rings/engines so the last parts don't
                        # serialize on one 2-deep ring
                        if tail and e % 2 == 1:
                            o_ps = psC_yt.tile([P, 512], F32, tag="yt", name="o_ps")
                        else:
                            o_ps = psC_scr.tile([P, 512], F32, tag="scr", name="o_ps")
                        for h in range(HG):
                            nc.tensor.matmul(
                                o_ps,
                                yT[:, h, t0 : t0 + P],
                                wp_sb[:, h, e * 512 : (e + 1) * 512],
                                start=(h == 0),
                                stop=(h == HG - 1),
                            )
                        dst = o_sb[:, e * 512 : (e + 1) * 512]
                        # GPSIMD can't read PSUM; split copies across ACT/DVE
                        if e == 1:
                            nc.vector.tensor_copy(dst, o_ps)
                        else:
                            nc.scalar.copy(dst, o_ps)
                        if last:
                            # final part: DMA each quarter as soon as it's
                            # assembled so nothing gates the drain
                            nc.sync.dma_start(
                                out[t0 : t0 + P, e * 512 : (e + 1) * 512], dst
                            )
                        elif tail and e == 1:
                            # half-row DMA as soon as it's assembled so the
                            # drain tail isn't gated on one full-row transfer
                            nc.sync.dma_start(
                                out[t0 : t0 + P, 0:1024], o_sb[:, 0:1024]
                            )
                    if last:
                        pass
                    elif tail:
                        nc.sync.dma_start(
                            out[t0 : t0 + P, 1024:2048], o_sb[:, 1024:2048]
                        )
                    else:
                        nc.sync.dma_start(out[t0 : t0 + P, :], o_sb)

                def finish(pend):
                    """Denominator matmuls + reciprocal + normalize. The
                    full-width chunks come tree-summed in exa[:,0]; the
                    diagonal chunks (the dominant, near-diagonal exp values)
                    accumulate via the f32 PSUM path — folding them into the
                    bf16 tree was measured to double the output error."""
                    Ti, h, yt_ps, exa = pend
                    tt0 = Ti * 512
                    den_ps = psC_scr.tile([P, 512], F32, tag="scr", name="den_ps")
                    nc.tensor.matmul(
                        den_ps, onesb, exa[:, 0, :], start=True, stop=False
                    )
                    for r in range(1, 4):
                        off = 128 * r
                        nc.tensor.matmul(
                            den_ps[:, off:512],
                            onesb,
                            exa[:, 4 * Ti + r, off:512],
                            start=False,
                            stop=(r == 3),
                        )
                    denr = cfin.tile([P, 512], F32, tag="denr")
                    nc.vector.reciprocal_approx_fast(denr, den_ps)
                    nc.vector.tensor_tensor(
                        yT[:, h, tt0 : tt0 + 512], yt_ps, denr, ALU.mult
                    )

                # proj quarter-blocks assigned to (Ti, h) slots; a part
                # (Tb, p) may only appear once finish(Tb, 3) has been emitted
                PARTS = {
                    (2, 0): [(0, 0), (0, 1)], (2, 1): [(0, 2), (0, 3)],
                    (2, 2): [(1, 0), (1, 1)], (2, 3): [(1, 2), (1, 3)],
                    (3, 0): [(2, 0)], (3, 1): [(2, 1)],
                    (3, 2): [(2, 2)], (3, 3): [(2, 3)],
                }
                pend = None
                for Ti in range(2, 4):
                    tt0 = Ti * 512
                    nfull = 4 * Ti + 1  # full-width chunks (incl. diag r=0)
                    for h in range(HG):
                        exa = cwork.tile([P, NT, 512], BF, tag="ex")
                        yt_ps = psC_yt.tile([P, 512], F32, tag="yt")

                        # full-width chunks, exp'd in pairs
                        S = 0
                        while S < nfull:
                            npair = 2 if S + 1 < nfull else 1
                            sc_ps = psC_sc.tile([P, 2, 512], F32, tag="sc")
                            for j in range(npair):
                                nc.tensor.matmul(
                                    sc_ps[:, j, :],
                                    kT[:, (S + j) * P : (S + j + 1) * P],
                                    qT[:, h, tt0 : tt0 + 512],
                                    start=True,
                                    stop=True,
                                )
                            nc.scalar.activation(
                                exa[:, S : S + npair, :],
                                sc_ps[:, 0:npair, :],
                                AF.Exp,
                                scale=SCALE,
                            )
                            if S + npair == nfull:
                                # diag r=0 staircase mask on leading 128 cols
                                nc.vector.tensor_tensor(
                                    exa[:, nfull - 1, 0:P],
                                    exa[:, nfull - 1, 0:P],
                                    dm1_sb,
                                    ALU.mult,
                                )
                            for j in range(npair):
                                nc.tensor.matmul(
                                    yt_ps,
                                    v_sb[:, S + j, :],
                                    exa[:, S + j, :],
                                    start=(S + j == 0),
                                    stop=False,
                                )
                            S += npair
                        # diagonal chunks r=1..3 (narrowing staircase); r=1,2
                        # share one PSUM pair and one exp over the union
                        # region (r=2's [128:256) stripe is unread garbage)
                        S0 = 4 * Ti
                        sc_ps = psC_sc.tile([P, 2, 512], F32, tag="sc")
                        for r in (1, 2):
                            off = 128 * r
                            nc.tensor.matmul(
                                sc_ps[:, r - 1, off:512],
                                kT[:, (S0 + r) * P : (S0 + r + 1) * P],
                                qT[:, h, tt0 + off : tt0 + 512],
                                start=True,
                                stop=True,
                            )
                        nc.scalar.activation(
                            exa[:, S0 + 1 : S0 + 3, 128:512],
                            sc_ps[:, 0:2, 128:512],
                            AF.Exp,
                            scale=SCALE,
                        )
                        sc_ps3 = psC_sc.tile([P, 2, 512], F32, tag="sc")
                        nc.tensor.matmul(
                            sc_ps3[:, 0, 384:512],
                            kT[:, (S0 + 3) * P : (S0 + 4) * P],
                            qT[:, h, tt0 + 384 : tt0 + 512],
                            start=True,
                            stop=True,
                        )
                        nc.scalar.activation(
                            exa[:, S0 + 3, 384:512], sc_ps3[:, 0, 384:512],
                            AF.Exp, scale=SCALE,
                        )
                        for r in range(1, 4):
                            S = S0 + r
                            off = 128 * r
                            nc.vector.tensor_tensor(
                                exa[:, S, off : off + P],
                                exa[:, S, off : off + P],
                                dm1_sb,
                                ALU.mult,
                            )
                            nc.tensor.matmul(
                                yt_ps[:, off:512],
                                v_sb[:, S, :],
                                exa[:, S, off:512],
                                start=False,
                                stop=(r == 3),
                            )
                        # binary-tree sum of the full-width chunks into
                        # exa[:,0], all on DVE
                        gap = 1
                        while gap < nfull:
                            for k in range(0, nfull - gap, 2 * gap):
                                nc.vector.tensor_tensor(
                                    exa[:, k, :], exa[:, k, :],
                                    exa[:, k + gap, :], ALU.add,
                                )
                            gap *= 2
                        if pend is not None:
                            finish(pend)
                        pend = (Ti, h, yt_ps, exa)
                        # projection quarter-blocks per head slot keep PE fed
                        # under the exp stream
                        for Tb, part in PARTS.get((Ti, h), []):
                            emit_proj_part(Tb, part)
                finish(pend)
                for part in range(4):
                    emit_proj_part(3, part, tail=True, last=(part == 3))

    nc.finalize()
    return nc


def _host_tables(q_norm_w, k_norm_w):
    """Packed RoPE cos/sin tables [T, 4, HD] (csq, snq, csk, snk) in bf16,
    with the q/k norm weights folded in."""
    half = HD // 2
    inv_freq = (
        1.0 / (ROPE_BASE ** (np.arange(0, half, dtype=np.float32) / half))
    ).astype(np.float32)
    ang = np.arange(T, dtype=np.float32)[:, None] * inv_freq[None, :]  # [T, half]
    cos = np.cos(ang).astype(np.float32)
    sin = np.sin(ang).astype(np.float32)
    cos2 = np.concatenate([cos, cos], axis=1)           # [T, 128]
    sin2 = np.concatenate([-sin, sin], axis=1)          # [T, 128]
    tabs = np.stack(
        [
            cos2 * q_norm_w[None, :],
            sin2 * q_norm_w[None, :],
            cos2 * k_norm_w[None, :],
            sin2 * k_norm_w[None, :],
        ],
        axis=1,
    )  # [T, 4, 128]
    return np.ascontiguousarray(tabs.astype(BF_NP))


def _host_masks():
    idx = np.arange(P)
    stair = (idx[None, :] // BLOCK >= idx[:, None] // BLOCK).astype(np.float32)
    return np.ascontiguousarray(stair.astype(BF_NP))


def prep_core_inputs(x, W_attn, W_proj, q_norm_w, k_norm_w):
    """Build the 8 per-core input maps (host-side sharding + dtype prep)."""
    x = np.asarray(x, dtype=np.float32)
    W_attn = np.asarray(W_attn, dtype=np.float32)
    W_proj = np.asarray(W_proj, dtype=np.float32)
    q_norm_w = np.asarray(q_norm_w, dtype=np.float32)
    k_norm_w = np.asarray(k_norm_w, dtype=np.float32)

    tabs = _host_tables(q_norm_w, k_norm_w)
    dm1 = _host_masks()
    xts = [
        np.ascontiguousarray(x[b].T.astype(BF_NP)) for b in range(x.shape[0])
    ]

    in_maps = []
    for core in range(8):
        b, g = divmod(core, 4)
        wa_core = np.concatenate(
            [
                W_attn[:, g * QCOLS : (g + 1) * QCOLS],
                W_attn[:, C + g * HD : C + (g + 1) * HD],
                W_attn[:, C + N_KV * HD + g * HD : C + N_KV * HD + (g + 1) * HD],
            ],
            axis=1,
        )
        wp_core = W_proj[g * QCOLS : (g + 1) * QCOLS, :]
        in_maps.append(
            {
                "xt": xts[b],
                "wa": np.ascontiguousarray(wa_core.astype(BF_NP)),
                "wp": np.ascontiguousarray(wp_core.astype(BF_NP)),
                "tabs": tabs,
                "dm1": dm1,
            }
        )
    return in_maps


_nc_cache = None


def kernel(x, W_attn, W_proj, q_norm_w, k_norm_w):
    global _nc_cache
    x = np.asarray(x, dtype=np.float32)
    B = x.shape[0]

    in_maps = prep_core_inputs(x, W_attn, W_proj, q_norm_w, k_norm_w)

    if _nc_cache is None:
        _nc_cache = build_nc()
    res = run_bass_kernel_spmd(_nc_cache, in_maps, core_ids=list(range(8)))

    out = np.zeros((B, T, C), dtype=np.float32)
    for core in range(8):
        b = core // 4
        out[b] += res.results[core]["out"]
    return out


# revision 82
# speedup vs baseline: 1.8916x; 1.8916x over previous
"""Block-causal GQA attention layer on 8 Trainium2 NeuronCores.

Sharding: 8 cores = batch(2) x head-group(4). Core c handles batch b=c//4 and
head group g=c%4 (q heads 4g..4g+3, kv head g). W_attn is column-sharded by
head group, W_proj row-sharded; each core computes a partial [T, C] output and
the host sums the 4 partials per batch element.

All on-device tensors are bf16 (matmuls run 1 cycle/row at any width; DVE gets
2x on 16-bit; DMA traffic halves) except PSUM accumulators and the final
output, which stay f32. x is pre-transposed on the host (xT [C,T]) so the
kernel needs no PE transposes for the QKV input. RoPE cos/sin tables (with the
q/k norm weights folded in) are packed [T, 4, HD] so each chunk is one DMA.

Schedule (single PE stream, engines pipelined via Tile deps):
  B) 16 t-chunks: iter i does xT/table DMAs, QKV matmuls into PSUM, a bf16
     SBUF copy of q/k (ACT; DVE for the first chunks while wa dma_starts
     clog the ACT sequencer), RMS square-sums (ACT Square with accum), and an
     all-DVE rsqrt (reciprocal_approx_fast seed + 2 Newton steps — keeping
     Sqrt off ACT holds the activation table on the one set that also has
     Exp, avoiding ~1.3us reloads at every interleave point). Lag-1 RoPE
     (Pool+DVE, rs applied by a final per-head tensor_scalar), lag-2 PE
     transposes of q/k into bf16 PSUM, DVE copyback into qT/kT.
     The Ti=0 block and all of Ti=1 interleave into iters 9..15 + epilogue:
     they are exp(ACT)-bound and fill B's ACT slack, and the (1,2)/(1,3)
     blocks keep PE busy through the chunk-15 drain (no chunk-15 deps).
  C) Ti=2,3 x head h: block-causal scores (full-width chunks exp'd in PSUM
     pairs, diag r=1,2 share a pair via a union-region exp; exact 128-wide
     staircase offsets), mask multiplies on the leading 128 cols, PV
     accumulation; softmax denominator = DVE binary-tree sum of the
     full-width exp chunks + one ones-matmul, plus 3 narrow f32 ones-matmuls
     for the dominant diagonal chunks (keeps output error at 6.8e-3),
     reciprocal_approx_fast, normalize into bf16 yT (finish lagged one head
     so PE never waits on the tree). Projection quarter-blocks interleave
     into every head slot (2 per Ti=2 slot, 1 per Ti=3 slot, 4 in the drain
     tail); PSUM->SBUF copies split 3:1 ACT:DVE (GPSIMD cannot access
     PSUM), one 8KB-run out DMA per t-chunk, split finer in the drain tail.

DMA queues: xT + out on SP, wa + wp on ACT, tables + mask on gpsimd SWDGE —
a single queue's ~1.3us per-dma_start sequencer cost would starve the rest.
"""

import numpy as np
import ml_dtypes

import concourse.bacc as bacc
import concourse.bass as bass
import concourse.tile as tile
import concourse.mybir as mybir
from concourse.bass_utils import run_bass_kernel_spmd
from concourse.masks import make_identity

P = 128
T = 2048
C = 2048
N_HEAD = 16
N_KV = 4
HD = 128          # head dim
HG = N_HEAD // N_KV  # heads per group = 4
BLOCK = 16
EPS = 1e-5
ROPE_BASE = 500000.0
QCOLS = HG * HD   # 512 q cols per core
JCOLS = QCOLS + 2 * HD  # 768 qkv cols per core
NT = T // P       # 16 t-chunks
NC16 = C // P     # 16 c-chunks
NX = 8            # xT DMA chunks (256 t-cols each)
SCALE = 1.0 / float(np.sqrt(np.float32(HD)))

F32 = mybir.dt.float32
BF = mybir.dt.bfloat16
AF = mybir.ActivationFunctionType
ALU = mybir.AluOpType
BF_NP = ml_dtypes.bfloat16


def build_nc():
    nc = bacc.Bacc("TRN2", target_bir_lowering=False)

    xt = nc.dram_tensor("xt", [C, T], BF, kind="ExternalInput")
    wa = nc.dram_tensor("wa", [C, JCOLS], BF, kind="ExternalInput")
    wp = nc.dram_tensor("wp", [QCOLS, C], BF, kind="ExternalInput")
    tabs = nc.dram_tensor("tabs", [T, 4, HD], BF, kind="ExternalInput")
    dm1 = nc.dram_tensor("dm1", [P, P], BF, kind="ExternalInput")
    out = nc.dram_tensor("out", [T, C], F32, kind="ExternalOutput")

    with tile.TileContext(nc) as tc:
        with tc.tile_pool(name="persist", bufs=1) as persist:
            ident_f = persist.tile([P, P], F32)
            make_identity(nc, ident_f)
            identb = persist.tile([P, P], BF)
            nc.vector.tensor_copy(identb, ident_f)
            ones_f = persist.tile([P, P], F32)
            nc.vector.memset(ones_f, 1.0)
            onesb = persist.tile([P, P], BF)
            nc.vector.tensor_copy(onesb, ones_f)
            dm1_sb = persist.tile([P, P], BF)
            nc.gpsimd.dma_start(dm1_sb, dm1[:])
            eps_sb = persist.tile([P, 1], F32)
            nc.vector.memset(eps_sb, EPS)

            qT = persist.tile([P, HG, T], BF)     # [d, h, t]
            kT = persist.tile([P, T], BF)         # [d, t]
            v_sb = persist.tile([P, NT, HD], BF)  # [s_in_chunk, s_chunk, d']
            yT = persist.tile([P, HG, T], BF)     # [d', h, t]
            wp_sb = persist.tile([P, HG, C], BF)

            # ---------------- Phase B (pipelined, lag 1) + C(Ti=0) --------
            with (
                tc.tile_pool(name="psB_qa", bufs=2, space="PSUM") as psB_qa,
                tc.tile_pool(name="psB_qb", bufs=2, space="PSUM") as psB_qb,
                tc.tile_pool(name="psC0_sc", bufs=2, space="PSUM") as psC0_sc,
                tc.tile_pool(name="psC0_yt", bufs=1, space="PSUM") as psC0_yt,
                tc.tile_pool(name="psB_tp", bufs=1, space="PSUM") as psB_tp,
                tc.tile_pool(name="wts", bufs=1) as wts,
                tc.tile_pool(name="bstream", bufs=2) as bstream,
                tc.tile_pool(name="tstream", bufs=4) as tstream,
                tc.tile_pool(name="bwork", bufs=4) as bwork,
                tc.tile_pool(name="c0work", bufs=2) as c0work,
            ):
                half = HD // 2
                st = {}        # chunk index -> per-chunk state dict
                x_tiles = {}   # xchunk index -> [P, NC16, 2P] tile

                xt_r = xt[:].rearrange("(ci p) t -> p ci t", p=P)

                def dma_xw(j, split=False):
                    xw = bstream.tile([P, NC16, 2 * P], BF, tag="xw")
                    if split:  # halve first-chunk latency
                        nc.sync.dma_start(
                            xw[:, 0:8, :], xt_r[:, 0:8, j * 2 * P : (j + 1) * 2 * P]
                        )
                        nc.sync.dma_start(
                            xw[:, 8:16, :], xt_r[:, 8:16, j * 2 * P : (j + 1) * 2 * P]
                        )
                    else:
                        nc.sync.dma_start(xw, xt_r[:, :, j * 2 * P : (j + 1) * 2 * P])
                    x_tiles[j] = xw

                wa_r = wa[:].rearrange("(co ci) j -> ci co j", ci=P)
                wa_tiles = []

                def load_wa():
                    for ci in range(NC16):
                        wa_ci = wts.tile([P, JCOLS], BF, tag=f"wa{ci}", name=f"wa{ci}")
                        nc.scalar.dma_start(wa_ci, wa_r[:, ci])
                        wa_tiles.append(wa_ci)

                def wa_tile(ci):
                    return wa_tiles[ci]

                tab_tiles = {}

                def dma_tab(i):
                    tab = tstream.tile([P, 4, HD], BF, tag="tab", name="tab_t")
                    nc.gpsimd.dma_start(tab, tabs[i * P : (i + 1) * P])
                    tab_tiles[i] = tab

                def stageB1(j):
                    """Pool/DVE rope for chunk j on the bf16 SBUF copy of the
                    QKV output (GPSIMD cannot touch PSUM and only supports
                    plain tensor_tensor); the RMS scale rs is applied by the
                    final per-head DVE tensor_scalar."""
                    s = st[j]
                    qsb, rs, tab = s["qsb"], s["rs"], s["tab"]

                    csq = tab[:, 0, :]
                    snq = tab[:, 1, :]
                    csk = tab[:, 2, :]
                    snk = tab[:, 3, :]
                    csq_b = bass.AP(
                        tensor=csq.tensor,
                        offset=csq.offset,
                        ap=[csq.ap[0], [0, HG], [1, HD]],
                    )
                    snq_b = bass.AP(
                        tensor=snq.tensor,
                        offset=snq.offset,
                        ap=[snq.ap[0], [0, HG], [half, 2], [1, half]],
                    )
                    qswp = bass.AP(
                        tensor=qsb.tensor,
                        offset=qsb.offset + half,
                        ap=[qsb.ap[0], [HD, HG], [-half, 2], [1, half]],
                    )
                    t1q = bwork.tile([P, QCOLS], BF, tag="t1q")
                    nc.gpsimd.tensor_tensor(
                        t1q.rearrange("p (h e) -> p h e", h=HG),
                        qsb[:, 0:QCOLS].rearrange("p (h e) -> p h e", h=HG),
                        csq_b,
                        ALU.mult,
                    )
                    t2q = bwork.tile([P, QCOLS], BF, tag="t2q")
                    nc.vector.tensor_tensor(
                        t2q.rearrange("p (h s e) -> p h s e", h=HG, s=2),
                        qswp,
                        snq_b,
                        ALU.mult,
                    )
                    qsum = bwork.tile([P, QCOLS], BF, tag="qsum")
                    nc.vector.tensor_tensor(qsum, t1q, t2q, ALU.add)
                    qhat = bwork.tile([P, QCOLS], BF, tag="qhat")
                    for hh in range(HG):
                        o = hh * HD
                        nc.vector.tensor_scalar_mul(
                            qhat[:, o : o + HD], qsum[:, o : o + HD],
                            rs[:, hh : hh + 1],
                        )

                    t1k = bwork.tile([P, HD], BF, tag="t1k")
                    nc.gpsimd.tensor_tensor(
                        t1k, qsb[:, QCOLS : QCOLS + HD], csk, ALU.mult
                    )
                    kswp = bass.AP(
                        tensor=qsb.tensor,
                        offset=qsb.offset + QCOLS + half,
                        ap=[qsb.ap[0], [-half, 2], [1, half]],
                    )
                    t2k = bwork.tile([P, HD], BF, tag="t2k")
                    nc.vector.tensor_tensor(
                        t2k.rearrange("p (s e) -> p s e", s=2),
                        kswp,
                        snk.rearrange("p (s e) -> p s e", s=2),
                        ALU.mult,
                    )
                    ksum = bwork.tile([P, HD], BF, tag="ksum")
                    nc.vector.tensor_tensor(ksum, t1k, t2k, ALU.add)
                    khat = bwork.tile([P, HD], BF, tag="khat")
                    nc.vector.tensor_scalar_mul(khat, ksum, rs[:, HG : HG + 1])
                    s["qhat"], s["khat"] = qhat, khat

                def stageB2(j):
                    """PE transposes of qhat/khat (bf16 PSUM) + copyback."""
                    s = st.pop(j)
                    t0 = j * P
                    tqk_ps = psB_tp.tile([P, 640], BF, tag="tp")
                    for hh in range(HG):
                        nc.tensor.transpose(
                            tqk_ps[:, hh * HD : (hh + 1) * HD],
                            s["qhat"][:, hh * HD : (hh + 1) * HD],
                            identb,
                        )
                    nc.tensor.transpose(tqk_ps[:, QCOLS : QCOLS + HD], s["khat"], identb)
                    nc.vector.tensor_copy(
                        qT[:, :, t0 : t0 + P],
                        tqk_ps[:, 0:QCOLS].rearrange("p (h t) -> p h t", h=HG),
                    )
                    nc.vector.tensor_copy(kT[:, t0 : t0 + P], tqk_ps[:, QCOLS:640])

                # --- early attention blocks (Ti=0 all h, Ti=1 h=0,1),
                # interleaved into B's tail iterations: they fill B's ACT/PE
                # slack, keep PE busy through the B epilogue, and shrink the
                # serial C region. Unpaired exps, c0 PSUM pools.
                c0state = {}

                def b_attn_block(Ti, h):
                    tt0 = Ti * 512
                    nS = 4 * Ti + 4
                    nfull = 4 * Ti + 1
                    exa = c0work.tile([P, 8, 512], BF, tag="c0ex")
                    yt_ps = psC0_yt.tile([P, 512], F32, tag="c0yt")
                    # scores emit one chunk ahead of PVs: the in-order PE
                    # queue would otherwise block the independent next-chunk
                    # scores behind a PV that waits on exp
                    prev = None
                    for S in range(nS):
                        r = S - 4 * Ti
                        off = 128 * r if r > 0 else 0
                        sc_ps = psC0_sc.tile([P, 512], F32, tag="c0sc")
                        nc.tensor.matmul(
                            sc_ps[:, off:512],
                            kT[:, S * P : (S + 1) * P],
                            qT[:, h, tt0 + off : tt0 + 512],
                            start=True,
                            stop=True,
                        )
                        nc.scalar.activation(
                            exa[:, S, off:512], sc_ps[:, off:512], AF.Exp,
                            scale=SCALE,
                        )
                        if r >= 0:
                            nc.vector.tensor_tensor(
                                exa[:, S, off : off + P],
                                exa[:, S, off : off + P],
                                dm1_sb,
                                ALU.mult,
                            )
                        if prev is not None:
                            Sp, offp = prev
                            nc.tensor.matmul(
                                yt_ps[:, offp:512],
                                v_sb[:, Sp, :],
                                exa[:, Sp, offp:512],
                                start=(Sp == 0),
                                stop=False,
                            )
                        prev = (S, off)
                    Sp, offp = prev
                    nc.tensor.matmul(
                        yt_ps[:, offp:512],
                        v_sb[:, Sp, :],
                        exa[:, Sp, offp:512],
                        start=(Sp == 0),
                        stop=True,
                    )
                    gap = 1
                    while gap < nfull:
                        for k in range(0, nfull - gap, 2 * gap):
                            nc.vector.tensor_tensor(
                                exa[:, k, :], exa[:, k, :],
                                exa[:, k + gap, :], ALU.add,
                            )
                        gap *= 2
                    c0state[(Ti, h)] = (exa, yt_ps)

                def b_attn_finish(Ti, h):
                    exa, yt_ps = c0state.pop((Ti, h))
                    tt0 = Ti * 512
                    den_ps = psC0_sc.tile([P, 512], F32, tag="c0sc", name="den0")
                    nc.tensor.matmul(den_ps, onesb, exa[:, 0, :], start=True, stop=False)
                    for r in range(1, 4):
                        off = 128 * r
                        nc.tensor.matmul(
                            den_ps[:, off:512],
                            onesb,
                            exa[:, 4 * Ti + r, off:512],
                            start=False,
                            stop=(r == 3),
                        )
                    denr = c0work.tile([P, 512], F32, tag="c0denr")
                    nc.vector.reciprocal_approx_fast(denr, den_ps)
                    nc.vector.tensor_tensor(
                        yT[:, h, tt0 : tt0 + 512], yt_ps, denr, ALU.mult
                    )

                dma_xw(0, split=True)
                dma_tab(0)
                load_wa()
                dma_tab(1)
                for i in range(NT + 1):
                    if i < NT:
                        if i % 2 == 0 and i // 2 + 1 < NX:
                            dma_xw(i // 2 + 1)
                        if i + 2 < NT:
                            dma_tab(i + 2)
                        if i == 8:
                            nc.scalar.dma_start(
                                wp_sb, wp[:].rearrange("(h d) e -> d h e", d=P)
                            )

                        # premult + rope for chunk i-1 (ACT/DVE overlap the MMs)
                        if i >= 1:
                            stageB1(i - 1)

                        # QKV matmuls: iter 0 emits chunks 0 AND 1 with the
                        # ci loops interleaved, so PE consumption of each wa
                        # tile (4 matmuls) matches its DMA arrival rate and
                        # the cold-start weight trickle never idles PE
                        def emit_qkv(chunks):
                            tiles = {}
                            for j in chunks:
                                sj = st.setdefault(j, {})
                                sj["tab"] = tab_tiles[j]
                                qa = psB_qa.tile([P, QCOLS], F32, tag="qa")
                                qb = psB_qb.tile([P, 2 * HD], F32, tag="qb")
                                sj["qa"], sj["qb"] = qa, qb
                                tiles[j] = (qa, qb)
                            for ci in range(NC16):
                                wac = wa_tile(ci)
                                for j in chunks:
                                    qa, qb = tiles[j]
                                    lhsT = x_tiles[j // 2][
                                        :, ci, (j % 2) * P : (j % 2) * P + P
                                    ]
                                    nc.tensor.matmul(
                                        qa, lhsT, wac[:, 0:QCOLS],
                                        start=(ci == 0), stop=(ci == NC16 - 1),
                                    )
                                    nc.tensor.matmul(
                                        qb, lhsT, wac[:, QCOLS:JCOLS],
                                        start=(ci == 0), stop=(ci == NC16 - 1),
                                    )

                        def emit_stats(j):
                            s = st[j]
                            qa_ps, qb_ps = s["qa"], s["qb"]
                            # v copy + bf16 SBUF copy of q/k (rope reads it
                            # from Pool, which cannot access PSUM) + square
                            # sums, on ACT (Square/Copy share the Exp
                            # act-table set; the first chunks' copies go to
                            # DVE — the wa dma_starts occupy the ACT
                            # sequencer for ~20us at cold start).
                            early = j < 3
                            NH1 = HG + 1
                            ss = bwork.tile([P, NH1], F32, tag="ss")
                            qsb = bwork.tile([P, QCOLS + HD], BF, tag="qsb")
                            if early:
                                nc.vector.tensor_copy(
                                    v_sb[:, j, :], qb_ps[:, HD : 2 * HD]
                                )
                                nc.vector.tensor_copy(qsb[:, 0:QCOLS], qa_ps)
                                nc.vector.tensor_copy(
                                    qsb[:, QCOLS : QCOLS + HD], qb_ps[:, 0:HD]
                                )
                            else:
                                nc.scalar.copy(
                                    v_sb[:, j, :], qb_ps[:, HD : 2 * HD]
                                )
                                nc.scalar.copy(qsb[:, 0:QCOLS], qa_ps)
                                nc.scalar.copy(
                                    qsb[:, QCOLS : QCOLS + HD], qb_ps[:, 0:HD]
                                )
                            s["qsb"] = qsb
                            for hh in range(NH1):
                                sq = bwork.tile([P, HD], F32, tag="sq")
                                src = (
                                    qa_ps[:, hh * HD : (hh + 1) * HD]
                                    if hh < HG
                                    else qb_ps[:, 0:HD]
                                )
                                nc.scalar.activation(
                                    sq, src, AF.Square,
                                    accum_out=ss[:, hh : hh + 1],
                                )
                            # rs = (eps + ss/HD)^-1/2 on DVE: reciprocal seed
                            # + 2 Newton steps (y' = y*(1.5 - 0.5*m*y^2)).
                            # Keeping Sqrt off ACT avoids act-table reloads at
                            # every B<->attention interleave point (~1.3us).
                            m_t = bwork.tile([P, NH1], F32, tag="m_t")
                            nc.vector.tensor_scalar(
                                m_t, ss, 1.0 / HD, EPS, ALU.mult, ALU.add
                            )
                            rr = bwork.tile([P, NH1], F32, tag="rr")
                            nc.vector.reciprocal_approx_fast(rr, m_t)
                            rs = bwork.tile([P, NH1], F32, tag="rs")
                            nc.vector.tensor_scalar(
                                rs, rr, 0.5, 0.5, ALU.mult, ALU.add
                            )
                            nr1 = bwork.tile([P, NH1], F32, tag="nr1")
                            nr2 = bwork.tile([P, NH1], F32, tag="nr2")
                            for _ in range(2):
                                nc.vector.tensor_tensor(nr1, m_t, rs, ALU.mult)
                                nc.vector.scalar_tensor_tensor(
                                    nr2, nr1, -0.5, rs, ALU.mult, ALU.mult
                                )
                                nc.vector.scalar_tensor_tensor(
                                    rs, nr2, 1.5, rs, ALU.add, ALU.mult
                                )
                            s["rs"] = rs

                        emit_qkv([i])
                        # chunk i-2 q/k transposes (lag 2: qhat is then
                        # guaranteed ready, PE never waits on rope)
                        if i >= 2:
                            stageB2(i - 2)
                        emit_stats(i)

                        # early attention interleave: all of Ti=0 and Ti=1
                        if 10 <= i <= 13:
                            b_attn_finish(0, i - 10)
                        elif i == 14:
                            b_attn_finish(1, 0)
                        elif i == 15:
                            b_attn_finish(1, 1)
                        if 9 <= i <= 12:
                            b_attn_block(0, i - 9)
                        elif i == 13:
                            b_attn_block(1, 0)
                        elif i == 14:
                            b_attn_block(1, 1)
                        elif i == 15:
                            b_attn_block(1, 2)
                    else:
                        # epilogue: chunk 15 rope/transposes interleaved with
                        # the (1,3) block (which has no chunk-15 dependency)
                        # so PE stays busy while ACT/DVE drain
                        stageB1(i - 1)
                        stageB2(i - 2)
                        b_attn_finish(1, 2)
                        b_attn_block(1, 3)
                        stageB2(i - 1)
                        b_attn_finish(1, 3)

            # ---------------- Phase C: Ti=1..3 + projection --------------
            with (
                tc.tile_pool(name="cwork", bufs=2) as cwork,
                tc.tile_pool(name="cfin", bufs=2) as cfin,
                tc.tile_pool(name="dout", bufs=3) as dout,
                tc.tile_pool(name="psC_yt", bufs=2, space="PSUM") as psC_yt,
                tc.tile_pool(name="psC_scr", bufs=2, space="PSUM") as psC_scr,
                tc.tile_pool(name="psC_sc", bufs=2, space="PSUM") as psC_sc,
            ):
                _pp = [0]

                def emit_proj_part(Tb, part, tail=False, last=False):
                    _pp[0] += 1
                    use_yt = (_pp[0] % 2 == 0)
                    tci = 4 * Tb + part
                    t0 = tci * P
                    o_sb = dout.tile([P, C], F32, tag="o_sb")
                    for e in range(4):
                        # in the drain tail, spread PSUM slots and copies
                        # across idle rings/engines so the last parts don't
                        # serialize on one 2-deep ring
                        if (tail and e % 2 == 1) or (not tail and use_yt and e % 2 == 1):
                            o_ps = psC_yt.tile([P, 512], F32, tag="yt", name="o_ps")
                        else:
                            o_ps = psC_scr.tile([P, 512], F32, tag="scr", name="o_ps")
                        for h in range(HG):
                            nc.tensor.matmul(
                                o_ps,
                                yT[:, h, t0 : t0 + P],
                                wp_sb[:, h, e * 512 : (e + 1) * 512],
                                start=(h == 0),
                                stop=(h == HG - 1),
                            )
                        dst = o_sb[:, e * 512 : (e + 1) * 512]
                        # GPSIMD can't read PSUM; split copies across ACT/DVE
                        if e == 1:
                            nc.vector.tensor_copy(dst, o_ps)
                        else:
                            nc.scalar.copy(dst, o_ps)
                        if last:
                            # final part: DMA each quarter as soon as it's
                            # assembled so nothing gates the drain
                            nc.sync.dma_start(
                                out[t0 : t0 + P, e * 512 : (e + 1) * 512], dst
                            )
                        elif tail and e == 1:
                            # half-row DMA as soon as it's assembled so the
                            # drain tail isn't gated on one full-row transfer
                            nc.sync.dma_start(
                                out[t0 : t0 + P, 0:1024], o_sb[:, 0:1024]
                            )
                    if last:
                        pass
                    elif tail:
                        nc.sync.dma_start(
                            out[t0 : t0 + P, 1024:2048], o_sb[:, 1024:2048]
                        )
                    else:
                        nc.sync.dma_start(out[t0 : t0 + P, :], o_sb)

                def finish(pend):
                    """Denominator matmuls + reciprocal + normalize. The
                    full-width chunks come tree-summed in exa[:,0]; the
                    diagonal chunks (the dominant, near-diagonal exp values)
                    accumulate via the f32 PSUM path — folding them into the
                    bf16 tree was measured to double the output error."""
                    Ti, h, yt_ps, exa = pend
                    tt0 = Ti * 512
                    den_ps = psC_scr.tile([P, 512], F32, tag="scr", name="den_ps")
                    nc.tensor.matmul(
                        den_ps, onesb, exa[:, 0, :], start=True, stop=False
                    )
                    for r in range(1, 4):
                        off = 128 * r
                        nc.tensor.matmul(
                            den_ps[:, off:512],
                            onesb,
                            exa[:, 4 * Ti + r, off:512],
                            start=False,
                            stop=(r == 3),
                        )
                    denr = cfin.tile([P, 512], F32, tag="denr")
                    nc.vector.reciprocal_approx_fast(denr, den_ps)
                    nc.vector.tensor_tensor(
                        yT[:, h, tt0 : tt0 + 512], yt_ps, denr, ALU.mult
                    )

                # proj quarter-blocks assigned to (Ti, h) slots; a part
                # (Tb, p) may only appear once finish(Tb, 3) has been emitted
                PARTS = {
                    (2, 0): [(0, 0), (0, 1)], (2, 1): [(0, 2), (0, 3)],
                    (2, 2): [(1, 0), (1, 1)], (2, 3): [(1, 2), (1, 3)],
                    (3, 0): [(2, 0)], (3, 1): [(2, 1)],
                    (3, 2): [(2, 2)], (3, 3): [(2, 3)],
                }
                pend = None
                for Ti in range(2, 4):
                    tt0 = Ti * 512
                    nfull = 4 * Ti + 1  # full-width chunks (incl. diag r=0)
                    for h in range(HG):
                        exa = cwork.tile([P, NT, 512], BF, tag="ex")
                        yt_ps = psC_yt.tile([P, 512], F32, tag="yt")

                        # full-width chunks, exp'd in pairs; scores emit one
                        # pair ahead of PVs (the in-order PE queue would
                        # otherwise block independent scores behind a PV
                        # that waits on exp)
                        S = 0
                        prevp = None
                        while S < nfull:
                            npair = 2 if S + 1 < nfull else 1
                            sc_ps = psC_sc.tile([P, 2, 512], F32, tag="sc")
                            for j in range(npair):
                                nc.tensor.matmul(
                                    sc_ps[:, j, :],
                                    kT[:, (S + j) * P : (S + j + 1) * P],
                                    qT[:, h, tt0 : tt0 + 512],
                                    start=True,
                                    stop=True,
                                )
                            nc.scalar.activation(
                                exa[:, S : S + npair, :],
                                sc_ps[:, 0:npair, :],
                                AF.Exp,
                                scale=SCALE,
                            )
                            if S + npair == nfull:
                                # diag r=0 staircase mask on leading 128 cols
                                nc.vector.tensor_tensor(
                                    exa[:, nfull - 1, 0:P],
                                    exa[:, nfull - 1, 0:P],
                                    dm1_sb,
                                    ALU.mult,
                                )
                            if prevp is not None:
                                for j in range(prevp[1]):
                                    nc.tensor.matmul(
                                        yt_ps,
                                        v_sb[:, prevp[0] + j, :],
                                        exa[:, prevp[0] + j, :],
                                        start=(prevp[0] + j == 0),
                                        stop=False,
                                    )
                            prevp = (S, npair)
                            S += npair
                        for j in range(prevp[1]):
                            nc.tensor.matmul(
                                yt_ps,
                                v_sb[:, prevp[0] + j, :],
                                exa[:, prevp[0] + j, :],
                                start=(prevp[0] + j == 0),
                                stop=False,
                            )
                        # diagonal chunks r=1..3 (narrowing staircase); r=1,2
                        # share one PSUM pair and one exp over the union
                        # region (r=2's [128:256) stripe is unread garbage)
                        S0 = 4 * Ti
                        sc_ps = psC_sc.tile([P, 2, 512], F32, tag="sc")
                        for r in (1, 2):
                            off = 128 * r
                            nc.tensor.matmul(
                                sc_ps[:, r - 1, off:512],
                                kT[:, (S0 + r) * P : (S0 + r + 1) * P],
                                qT[:, h, tt0 + off : tt0 + 512],
                                start=True,
                                stop=True,
                            )
                        nc.scalar.activation(
                            exa[:, S0 + 1 : S0 + 3, 128:512],
                            sc_ps[:, 0:2, 128:512],
                            AF.Exp,
                            scale=SCALE,
                        )
                        sc_ps3 = psC_sc.tile([P, 2, 512], F32, tag="sc")
                        nc.tensor.matmul(
                            sc_ps3[:, 0, 384:512],
                            kT[:, (S0 + 3) * P : (S0 + 4) * P],
                            qT[:, h, tt0 + 384 : tt0 + 512],
                            start=True,
                            stop=True,
                        )
                        nc.scalar.activation(
                            exa[:, S0 + 3, 384:512], sc_ps3[:, 0, 384:512],
                            AF.Exp, scale=SCALE,
                        )
                        for r in range(1, 4):
                            S = S0 + r
                            off = 128 * r
                            nc.vector.tensor_tensor(
                                exa[:, S, off : off + P],
                                exa[:, S, off : off + P],
                                dm1_sb,
                                ALU.mult,
                            )
                            nc.tensor.matmul(
                                yt_ps[:, off:512],
                                v_sb[:, S, :],
                                exa[:, S, off:512],
                                start=False,
                                stop=(r == 3),
                            )
                        # binary-tree sum of the full-width chunks into
                        # exa[:,0], all on DVE
                        gap = 1
                        while gap < nfull:
                            for k in range(0, nfull - gap, 2 * gap):
                                nc.vector.tensor_tensor(
                                    exa[:, k, :], exa[:, k, :],
                                    exa[:, k + gap, :], ALU.add,
                                )
                            gap *= 2
                        if pend is not None:
                            finish(pend)
                        pend = (Ti, h, yt_ps, exa)
                        # projection quarter-blocks per head slot keep PE fed
                        # under the exp stream
                        for Tb, part in PARTS.get((Ti, h), []):
                            emit_proj_part(Tb, part)
                finish(pend)
                for part in range(4):
                    emit_proj_part(3, part, tail=True, last=(part == 3))

    nc.finalize()
    return nc


def _host_tables(q_norm_w, k_norm_w):
    """Packed RoPE cos/sin tables [T, 4, HD] (csq, snq, csk, snk) in bf16,
    with the q/k norm weights folded in."""
    half = HD // 2
    inv_freq = (
        1.0 / (ROPE_BASE ** (np.arange(0, half, dtype=np.float32) / half))
    ).astype(np.float32)
    ang = np.arange(T, dtype=np.float32)[:, None] * inv_freq[None, :]  # [T, half]
    cos = np.cos(ang).astype(np.float32)
    sin = np.sin(ang).astype(np.float32)
    cos2 = np.concatenate([cos, cos], axis=1)           # [T, 128]
    sin2 = np.concatenate([-sin, sin], axis=1)          # [T, 128]
    tabs = np.stack(
        [
            cos2 * q_norm_w[None, :],
            sin2 * q_norm_w[None, :],
            cos2 * k_norm_w[None, :],
            sin2 * k_norm_w[None, :],
        ],
        axis=1,
    )  # [T, 4, 128]
    return np.ascontiguousarray(tabs.astype(BF_NP))


def _host_masks():
    idx = np.arange(P)
    stair = (idx[None, :] // BLOCK >= idx[:, None] // BLOCK).astype(np.float32)
    return np.ascontiguousarray(stair.astype(BF_NP))


def prep_core_inputs(x, W_attn, W_proj, q_norm_w, k_norm_w):
    """Build the 8 per-core input maps (host-side sharding + dtype prep)."""
    x = np.asarray(x, dtype=np.float32)
    W_attn = np.asarray(W_attn, dtype=np.float32)
    W_proj = np.asarray(W_proj, dtype=np.float32)
    q_norm_w = np.asarray(q_norm_w, dtype=np.float32)
    k_norm_w = np.asarray(k_norm_w, dtype=np.float32)

    tabs = _host_tables(q_norm_w, k_norm_w)
    dm1 = _host_masks()
    xts = [
        np.ascontiguousarray(x[b].T.astype(BF_NP)) for b in range(x.shape[0])
    ]

    in_maps = []
    for core in range(8):
        b, g = divmod(core, 4)
        wa_core = np.concatenate(
            [
                W_attn[:, g * QCOLS : (g + 1) * QCOLS],
                W_attn[:, C + g * HD : C + (g + 1) * HD],
                W_attn[:, C + N_KV * HD + g * HD : C + N_KV * HD + (g + 1) * HD],
            ],
            axis=1,
        )
        wp_core = W_proj[g * QCOLS : (g + 1) * QCOLS, :]
        in_maps.append(
            {
                "xt": xts[b],
                "wa": np.ascontiguousarray(wa_core.astype(BF_NP)),
                "wp": np.ascontiguousarray(wp_core.astype(BF_NP)),
                "tabs": tabs,
                "dm1": dm1,
            }
        )
    return in_maps


_nc_cache = None


def kernel(x, W_attn, W_proj, q_norm_w, k_norm_w):
    global _nc_cache
    x = np.asarray(x, dtype=np.float32)
    B = x.shape[0]

    in_maps = prep_core_inputs(x, W_attn, W_proj, q_norm_w, k_norm_w)

    if _nc_cache is None:
        _nc_cache = build_nc()
    res = run_bass_kernel_spmd(_nc_cache, in_maps, core_ids=list(range(8)))

    out = np.zeros((B, T, C), dtype=np.float32)
    for core in range(8):
        b = core // 4
        out[b] += res.results[core]["out"]
    return out
